# revision 1
# baseline (speedup 1.0000x reference)
"""Trainium2 Bass kernel for the BinaryMechanismSSM problem.

Full inputs in, full outputs out. Internally: batch (128) sharded 8 ways
(16 rows/core). Per core:
  Phase 1: projections bx0/bx1/gx = x @ {B0,B1,G}^T + bias (f32r matmuls,
           N=512 token tiles), sigmoid applied to the gate, staged to DRAM
           planes proj[mat][j] = [128, T*16] (token = t*16 + b).
  Phase 2: T sequential steps. State held as st[p, 16j+b] = s[b, 128j+p]
           ([128, 64] tile). Per step: 32 fp16 matmuls (weight-stationary
           A-blocks, rhs = fp16 state slices) accumulate f0/f1 into
           [128, 64] PSUM; DVE adds the staged projections; ACT tanh;
           DVE blend + gate; per-step DMA of the new state to a staging
           buffer [T, 128, 64]. Host re-layouts to [B, T+1, S].
"""
import numpy as np

B_FULL = 128
T_FULL = 1024
I_DIM = 256
S_DIM = 512
N_CORES = 8
B_LOC = B_FULL // N_CORES  # 16

_cache = {}


def _build(alpha: float, z: int, T: int):
    import concourse.bass as bass
    from concourse import bacc
    import concourse.mybir as mybir
    from concourse.tile import TileContext

    dt = mybir.dt
    AF = mybir.ActivationFunctionType
    ALU = mybir.AluOpType

    TOK = T * B_LOC          # tokens per core
    NTT = TOK // 512         # phase-1 token tiles
    NG = T // 16             # phase-2 step groups
    NMAT = 3 if z != 0 else 2          # number of projection matrices
    NREC = 2 if z != 0 else 1          # number of recurrence matrices

    nc = bacc.Bacc("TRN2", target_bir_lowering=False, debug=False,
                   num_devices=N_CORES)

    xT_d = nc.declare_dram_parameter("xT", [2, 128, TOK], dt.float32r, isOutput=False)
    pw_d = nc.declare_dram_parameter("pw", [128, NMAT * 2 * 4 * 128], dt.float32r, isOutput=False)
    bias_d = nc.declare_dram_parameter("bias", [128, 4 * NMAT], dt.float32, isOutput=False)
    aw_d = nc.declare_dram_parameter("aw", [128, NREC * 16 * 128], dt.float16, isOutput=False)
    s0_d = nc.declare_dram_parameter("s0T", [128, 64], dt.float32, isOutput=False)
    iden_d = nc.declare_dram_parameter("iden", [128, 128], dt.float16, isOutput=False)
    stg_d = nc.declare_dram_parameter("stg", [T, 128, 64], dt.float32, isOutput=True)

    with TileContext(nc) as tc:
      with tc.tile_pool(name="dram", bufs=1, space="DRAM") as dpool:
        projh_p = [[dpool.tile([128, TOK], dt.float16, tag=f"projh{m}{j}",
                               name=f"projh{m}{j}")
                    for j in range(4)] for m in range(NREC)]
        projl_p = [[dpool.tile([128, TOK], dt.float16, tag=f"projl{m}{j}",
                               name=f"projl{m}{j}")
                    for j in range(4)] for m in range(NREC)]
        projg_p = [dpool.tile([128, TOK], dt.float32, tag=f"projg{j}",
                              name=f"projg{j}") for j in range(4)]
        # ---------------- Phase 1: projections ----------------
        with (
            tc.tile_pool(name="p1w", bufs=1) as p1w,
            tc.tile_pool(name="p1x", bufs=3) as p1x,
            tc.tile_pool(name="p1o", bufs=6) as p1o,
            tc.tile_pool(name="p1ps", bufs=8, space="PSUM") as p1ps,
        ):
            pw = p1w.tile([128, NMAT * 2 * 4 * 128], dt.float32r)
            nc.sync.dma_start(pw[:], pw_d[:])
            bias = p1w.tile([128, 4 * NMAT], dt.float32)
            nc.sync.dma_start(bias[:], bias_d[:])

            for tt in range(NTT):
                xt = p1x.tile([128, 2 * 512], dt.float32r, tag="xt")
                for i in range(2):
                    nc.sync.dma_start(xt[:, i * 512:(i + 1) * 512],
                                      xT_d[i, :, tt * 512:(tt + 1) * 512])
                for mat in range(NMAT):
                    for j in range(4):
                        ps = p1ps.tile([128, 512], dt.float32, tag="pps")
                        for i in range(2):
                            blk = ((mat * 2 + i) * 4 + j) * 128
                            nc.tensor.matmul(
                                ps[:], pw[:, blk:blk + 128],
                                xt[:, i * 512:(i + 1) * 512],
                                start=(i == 0), stop=(i == 1))
                        bj = bias[:, mat * 4 + j:mat * 4 + j + 1]
                        if mat == NMAT - 1:
                            ot = p1o.tile([128, 512], dt.float32, tag="po")
                            nc.scalar.activation(ot[:], ps[:], AF.Sigmoid,
                                                 bias=bj, scale=1.0)
                            nc.sync.dma_start(
                                projg_p[j][:, tt * 512:(tt + 1) * 512], ot[:])
                        else:
                            hi = p1o.tile([128, 512], dt.float16, tag="phi")
                            nc.scalar.activation(hi[:], ps[:], AF.Identity,
                                                 bias=bj, scale=1.0)
                            lo = p1o.tile([128, 512], dt.float16, tag="plo")
                            nc.vector.scalar_tensor_tensor(
                                lo[:], ps[:], bj, hi[:], ALU.add, ALU.subtract)
                            nc.sync.dma_start(
                                projh_p[mat][j][:, tt * 512:(tt + 1) * 512], hi[:])
                            nc.sync.dma_start(
                                projl_p[mat][j][:, tt * 512:(tt + 1) * 512], lo[:])

        # ---------------- Phase 2: recurrence ----------------
        with (
            tc.tile_pool(name="p2w", bufs=1) as p2w,
            tc.tile_pool(name="p2in", bufs=2) as p2in,
            tc.tile_pool(name="p2st", bufs=2) as p2st,
            tc.tile_pool(name="p2c", bufs=3) as p2c,
            tc.tile_pool(name="p2ps", bufs=4, space="PSUM") as p2ps,
        ):
            aw = p2w.tile([128, NREC * 16 * 128], dt.float16)
            nc.sync.dma_start(aw[:], aw_d[:])
            iden = p2w.tile([128, 128], dt.float16)
            nc.sync.dma_start(iden[:], iden_d[:])

            st = p2st.tile([128, 64], dt.float32, tag="st")
            nc.sync.dma_start(st[:], s0_d[:])
            st16 = p2st.tile([128, 64], dt.float16, tag="st16")
            nc.scalar.activation(st16[:], st[:], AF.Copy)

            GATE_MAT = NMAT - 1
            a0 = float(1.0 - alpha) if z != 0 else 1.0
            a1 = float(alpha)

            for g in range(NG):
                # staged bx planes (f32r for the identity-MM injection)
                # contiguous hi/lo fp16 staging: (h, m, j, t, b)
                pjb = p2in.tile([128, 2 * NREC * 4 * 256], dt.float16, tag="pjb")
                for h, planes in enumerate((projh_p, projl_p)):
                    for m in range(NREC):
                        for j in range(4):
                            nc.sync.dma_start(
                                pjb[:, ((h * NREC + m) * 4 + j) * 256:
                                       ((h * NREC + m) * 4 + j + 1) * 256],
                                planes[m][j][:, g * 256:(g + 1) * 256])
                pjbr = pjb[:].rearrange("p (h m j t b) -> p h m j t b",
                                        h=2, m=NREC, j=4, t=16, b=16)
                # staged gate plane (fp32 for DVE)
                pjg = p2in.tile([128, 4 * 256], dt.float32, tag="pjg")
                for j in range(4):
                    nc.sync.dma_start(
                        pjg[:, j * 256:(j + 1) * 256],
                        projg_p[j][:, g * 256:(g + 1) * 256])

                # per-group gate coefficient planes (off the serial path):
                # gco[:, m-block] = coef_m * g ; g1m = 1 - g
                gco = p2in.tile([128, NREC * 1024], dt.float32, tag="gco")
                nc.vector.tensor_scalar_mul(gco[:, 0:1024], pjg[:], a0)
                if NREC == 2:
                    nc.vector.tensor_scalar_mul(gco[:, 1024:2048], pjg[:], a1)
                gcor = gco[:].rearrange("p (m j t b) -> p m j t b",
                                        m=NREC, j=4, t=16, b=16)
                g1m = p2in.tile([128, 1024], dt.float32, tag="g1m")
                nc.vector.tensor_scalar(g1m[:], pjg[:], -1.0, 1.0,
                                        ALU.mult, ALU.add)
                g1mr = g1m[:].rearrange("p (j t b) -> p j t b", j=4, t=16)

                for tt in range(16):
                    t = g * 16 + tt
                    W = NREC * 64
                    pscat = p2ps.tile([128, W], dt.float32, tag="pscat")
                    # inject bx = hi + lo via fp16 identity matmuls
                    for m in range(NREC):
                        for h in range(2):
                            nc.tensor.matmul(
                                pscat[:, m * 64:(m + 1) * 64]
                                .rearrange("p (j b) -> p j b", j=4),
                                iden[:], pjbr[:, h, m, :, tt, :],
                                start=(m == 0 and h == 0), stop=False)
                    # m2 = (1-g) * s  (off serial path, only needs st)
                    m2 = p2c.tile([128, 64], dt.float32, tag="m2")
                    nc.vector.tensor_tensor(
                        m2[:].rearrange("p (j b) -> p j b", j=4),
                        st[:].rearrange("p (j b) -> p j b", j=4),
                        g1mr[:, :, tt, :], ALU.mult)
                    # A matmuls accumulate on top
                    for m in range(NREC):
                        for j in range(4):
                            for k in range(4):
                                blk = (m * 16 + k * 4 + j) * 128
                                nc.tensor.matmul(
                                    pscat[:, (m * 4 + j) * 16:(m * 4 + j + 1) * 16],
                                    aw[:, blk:blk + 128],
                                    st16[:, k * 16:(k + 1) * 16],
                                    start=False,
                                    stop=(k == 3))
                    # one tanh over the whole [128, NREC*64] psum
                    ft = p2c.tile([128, W], dt.float32, tag="ft")
                    nc.scalar.activation(ft[:], pscat[:], AF.Tanh)
                    # mcat = gco_t * ft
                    mc = p2c.tile([128, W], dt.float32, tag="mc")
                    nc.vector.tensor_tensor(
                        mc[:].rearrange("p (m j b) -> p m j b", m=NREC, j=4),
                        ft[:].rearrange("p (m j b) -> p m j b", m=NREC, j=4),
                        gcor[:, :, :, tt, :], ALU.mult)
                    # reduce + new state (fp16 copy gates next step's matmuls)
                    if NREC == 2:
                        t2 = p2c.tile([128, 64], dt.float32, tag="t2")
                        nc.vector.tensor_tensor(t2[:], mc[:, 0:64], mc[:, 64:128],
                                                ALU.add)
                    else:
                        t2 = mc
                    st16_new = p2st.tile([128, 64], dt.float16, tag="st16")
                    nc.vector.tensor_tensor(st16_new[:], t2[:], m2[:], ALU.add)
                    st_new = p2st.tile([128, 64], dt.float32, tag="st")
                    nc.vector.tensor_tensor(st_new[:], t2[:], m2[:], ALU.add)
                    st, st16 = st_new, st16_new

                    nc.sync.dma_start(stg_d[t], st[:])

    nc.compile()
    return nc


def _pack_lhsT_blocks(W, kdim, mdim, dtype):
    """W: [mdim*128, kdim*128]; returns [128, kdim*mdim*128] with block
    (k, j) at cols (k*mdim+j)*128 equal to W[j-chunk, k-chunk].T."""
    nk, nj = kdim, mdim
    out = np.zeros((128, nk * nj * 128), dtype=dtype)
    for k in range(nk):
        for j in range(nj):
            blk = W[j * 128:(j + 1) * 128, k * 128:(k + 1) * 128].T
            out[:, (k * nj + j) * 128:(k * nj + j + 1) * 128] = blk
    return np.ascontiguousarray(out)


def kernel(x_seq, s0, A0_w, B0_w, B0_b, A1_w, B1_w, B1_b, gate_w, gate_b,
           alpha, z, _T=None, _trace=False):
    from concourse.bass_utils import run_bass_kernel_spmd

    T = int(_T or T_FULL)
    alpha_f = float(np.asarray(alpha))
    z_i = int(np.asarray(z))

    key = (alpha_f, z_i, T)
    if key not in _cache:
        _cache[key] = _build(alpha_f, z_i, T)
    nc = _cache[key]

    NMAT = 3 if z_i != 0 else 2
    NREC = 2 if z_i != 0 else 1

    x_seq = np.asarray(x_seq, dtype=np.float32)
    s0 = np.asarray(s0, dtype=np.float32)

    # ---- shared (replicated) weight packing ----
    # pw: phase-1 lhsT blocks per matrix: (mat, i, j) at col ((mat*2+i)*4+j)*128
    mats = [np.asarray(B0_w), np.asarray(B1_w), np.asarray(gate_w)][:NMAT] \
        if z_i != 0 else [np.asarray(B0_w), np.asarray(gate_w)]
    biases = [np.asarray(B0_b), np.asarray(B1_b), np.asarray(gate_b)][:NMAT] \
        if z_i != 0 else [np.asarray(B0_b), np.asarray(gate_b)]
    pw = np.concatenate(
        [_pack_lhsT_blocks(W.astype(np.float32), 2, 4, np.float32).reshape(128, 2, 4 * 128).reshape(128, -1)
         for W in mats], axis=1)
    # note: _pack_lhsT_blocks already gives (i*4+j) ordering per matrix
    pw = np.ascontiguousarray(pw)

    bias = np.zeros((128, 4 * NMAT), np.float32)
    for mi, bvec in enumerate(biases):
        bias[:, mi * 4:(mi + 1) * 4] = bvec.astype(np.float32).reshape(4, 128).T

    recs = [np.asarray(A0_w)] if z_i == 0 else [np.asarray(A0_w), np.asarray(A1_w)]
    aw = np.concatenate(
        [_pack_lhsT_blocks(A.astype(np.float32), 4, 4, np.float32)
         for A in recs], axis=1).astype(np.float16)
    aw = np.ascontiguousarray(aw)

    IDEN = np.ascontiguousarray(np.eye(128, dtype=np.float16))

    # ---- per-core inputs ----
    in_maps = []
    for c in range(N_CORES):
        bc = c * B_LOC
        xc = x_seq[bc:bc + B_LOC, :T]                       # [16, T, 256]
        xT = np.ascontiguousarray(
            xc.transpose(2, 1, 0).reshape(2, 128, T * B_LOC))
        s0c = s0[bc:bc + B_LOC]                             # [16, 512]
        s0T = np.ascontiguousarray(
            s0c.T.reshape(4, 128, B_LOC).transpose(1, 0, 2).reshape(128, 64))
        in_maps.append({
            "xT": xT, "pw": pw, "bias": bias, "aw": aw, "s0T": s0T,
            "iden": IDEN,
        })

    res = run_bass_kernel_spmd(nc, in_maps, list(range(N_CORES)), trace=_trace)
    if _trace:
        kernel._last_res = res

    out = np.empty((B_FULL, T + 1, S_DIM), np.float32)
    for c in range(N_CORES):
        bc = c * B_LOC
        stg = res.results[c]["stg"]                         # [T, 128, 64]
        out[bc:bc + B_LOC, 0] = s0[bc:bc + B_LOC]
        out[bc:bc + B_LOC, 1:] = (
            stg.reshape(T, 128, 4, B_LOC).transpose(3, 0, 2, 1)
            .reshape(B_LOC, T, S_DIM))
    return out

